# revision 6
# baseline (speedup 1.0000x reference)
"""Trainium2 Bass kernel for a dense fp32 MultiHeadAttention layer.

Problem (hardcoded): B=4, T=S=2048, C=1024, 16 heads x 64 dims, fp32.
  q = query @ Wq.T + bq ; k,v likewise
  scores = (q k^T) * D**-0.5 + attn_mask + padding_mask
  out = softmax(scores) @ v -> reshape -> @ Wout.T + bout

Sharding over 8 NeuronCores: core c = (batch b = c//2, head-group g = c%2).
Each core handles one batch and 8 of the 16 heads:
  - column-parallel q/k/v projections (512-dim slice of the projections)
  - attention for its 8 heads (full T x S, on-chip scores)
  - row-parallel out_proj producing a partial (T, C) output
Host sums the two partials per batch and adds the bias terms
(bout + bv @ Wout.T, which commutes with softmax since sum(weights)=1).

Layout notes (per core):
  - host ships transposed activations xT (C, T) so projections need no
    on-device transposes:
      qT/kT (f-major): psum = wT_chunk.T @ xT_chunk   (f on partitions)
      v (s-major):     psum = xT_chunk.T @ wT_chunk   (s on partitions)
  - scores are computed transposed, (s on partitions, t free):
      psc = kT_chunk.T @ qT   so softmax's s-reduction becomes a matmul
  - v is stored with a ones column per head (65 wide); the PV matmul
      outT = [v|1].T @ exp(scores^T)
    then yields numerator rows 0..63 and the softmax denominator in row 64.
  - normalization: recip of row 64, broadcast across partitions with a
    one-hot matmul (e1.T @ recip_row), then one DVE multiply.

Schedule (phase 2): a flat software-pipelined stream over 16 head-pair
"blocks" (4 t-chunks x 4 pairs).  Block k interleaves, per s-chunk:
  scores(k, sc) -> PV(k-1, sc) -> exp(k, sc)
so the Tensor engine always has exp-independent PV work while the Scalar
engine (the pacing engine, ~1.04us per 1024-wide exp) chews through the
previous scores.  The softmax normalization for pair k-1 is split:
  - block k tail: denominator recip chain (DVE) + raw attnT copy
    (frees the PV psum bank immediately)
  - block k+1 head: e1-broadcast matmul + in-place attnT multiply
so the Tensor engine never waits on the DVE recip latency.  q/out
projections are spread into fixed blocks (2/6/10 and 5,7/9,11/13,14/17)
that never mix in one block, keeping the 2-slot pgen PSUM ring
deadlock-free.  exp values and v are stored bf16 (only used by the PV
matmul, 1 cyc/row either way) to halve their SBUF footprint.
"""

import os
import numpy as np

import concourse.bass as bass
import concourse.mybir as mybir
import concourse.tile as tile
from concourse import bacc
from concourse.bass_utils import run_bass_kernel_spmd

# ---- problem constants ----
B, T, S, C = 4, 2048, 2048, 1024
H, D = 16, 64
NCORES = 8
F = 512            # per-core projection slice (8 heads x 64)
SCALE = D ** -0.5
P = 128
TCH = 512          # t-chunk (score free dim)
NTC = T // TCH     # 4
NSC = S // P       # 16 s-chunks
NFC = F // P       # 4 f-chunks per core
NCC = C // P       # 8 contraction chunks
HW = 65            # v width per head incl. ones column
NBLK = NTC * NFC   # 16 head-pair blocks

FP32 = mybir.dt.float32
BF16 = mybir.dt.bfloat16

# matmul dtype knobs (float32 = exact 4cyc/row, float32r = 1cyc/row reduced precision)
MM_DT = getattr(mybir.dt, os.environ.get("MHA_MM_DT", "float32r"))
BC_DT = getattr(mybir.dt, os.environ.get("MHA_BC_DT", "float32r"))

LAST_EXEC_NS = None
LAST_TRACE = None
LAST_NC = None
LAST_IN_MAPS = None


DT_MM = MM_DT  # dtype for all matmul-operand tensors/tiles

# block -> (target tcx, fcp) for the q projection; never in an outproj block
QPLAN = {2: (1, 0), 6: (2, 0), 10: (3, 0)}
# block -> (tcx, [groups]) for the out projection (group g: tw=g//2, fh=g%2)
OPLAN = {5: (0, [0, 1, 2, 3]), 7: (0, [4, 5, 6, 7]),
         9: (1, [0, 1, 2, 3]), 11: (1, [4, 5, 6, 7]),
         13: (2, [0, 1, 2, 3]), 14: (2, [4, 5, 6, 7]),
         17: (3, [0, 1, 2, 3, 4, 5, 6, 7])}


def build(use_mask: bool):
    nc = bacc.Bacc("TRN2", target_bir_lowering=False, debug=False,
                   num_devices=NCORES)

    # exp/v dtype: bf16 when maskless (saves SBUF; PV matmul speed is the
    # same); keep fp32r in the mask variant so the mask multiply stays
    # same-dtype.
    EXP_DT = DT_MM if use_mask else BF16

    xq = nc.dram_tensor("xq", [C, T], DT_MM, kind="ExternalInput")
    xk = nc.dram_tensor("xk", [C, S], DT_MM, kind="ExternalInput")
    xv = nc.dram_tensor("xv", [C, S], DT_MM, kind="ExternalInput")
    wq = nc.dram_tensor("wq", [C, F], DT_MM, kind="ExternalInput")
    wk = nc.dram_tensor("wk", [C, F], DT_MM, kind="ExternalInput")
    wv = nc.dram_tensor("wv", [C, F], DT_MM, kind="ExternalInput")
    wo = nc.dram_tensor("wo", [F, C], DT_MM, kind="ExternalInput")
    bqr = nc.dram_tensor("bqr", [P, NFC], FP32, kind="ExternalInput")
    bkr = nc.dram_tensor("bkr", [P, NFC], FP32, kind="ExternalInput")
    if use_mask:
        emask = nc.dram_tensor("emask", [S, T], FP32, kind="ExternalInput")
    out = nc.dram_tensor("out", [T, C], FP32, kind="ExternalOutput")

    xq_r = xq.rearrange("(cc p) t -> p cc t", p=P)
    xk_r = xk.rearrange("(cc p) s -> p cc s", p=P)
    xv_r = xv.rearrange("(cc p) s -> p cc s", p=P)
    wq_r = wq.rearrange("(cc p) f -> p cc f", p=P)
    wk_r = wk.rearrange("(cc p) f -> p cc f", p=P)
    wv_r = wv.rearrange("(cc p) f -> p cc f", p=P)
    wo_r = wo.rearrange("(dc p) f -> p dc f", p=P)

    with tile.TileContext(nc) as tc:
        with tc.tile_pool(name="const", bufs=1) as cp:
            wq_sb = cp.tile([P, NCC, F], DT_MM, tag="wq")
            wo_sb = cp.tile([P, NFC, C], DT_MM, tag="wo")
            bq_sb = cp.tile([P, NFC], FP32, tag="bq")
            bk_sb = cp.tile([P, NFC], FP32, tag="bk")
            e1_sb = cp.tile([P, D], BC_DT, tag="e1")
            rrow_sb = [cp.tile([P, 2 * TCH], BC_DT, tag=f"rrow{i}",
                               name=f"rrow{i}") for i in range(2)]
            rr32_sb = cp.tile([1, 2 * TCH], FP32, tag="rr32")
            rr32b_sb = cp.tile([1, 2 * TCH], FP32, tag="rr32b")
            kT_sb = cp.tile([P, NFC, S], DT_MM, tag="kT")
            v_sb = cp.tile([P, NSC, 8 * HW], EXP_DT, tag="v")

            nc.sync.dma_start(bq_sb[:], bqr[:])
            nc.sync.dma_start(bk_sb[:], bkr[:])
            # fp32r tiles can't be memset directly; broadcast-copy from fp32
            # scratch columns instead (DVE rounds on write, exact for 0/1).
            one_sb = cp.tile([P, 1], FP32, tag="one")
            zero_sb = cp.tile([P, 1], FP32, tag="zero")
            nc.any.memset(one_sb[:], 1.0)
            nc.any.memset(zero_sb[:], 0.0)
            nc.vector.tensor_copy(e1_sb[:], zero_sb[:, 0:1].to_broadcast(e1_sb.shape))
            nc.vector.tensor_copy(e1_sb[0:1, :],
                                  one_sb[0:1, 0:1].to_broadcast((1, D)))
            for i in range(2):
                nc.vector.tensor_copy(
                    rrow_sb[i][:],
                    zero_sb[:, 0:1].to_broadcast(rrow_sb[i].shape))
            ones_dst = v_sb[:].rearrange("p s (h e) -> p s h e", e=HW)[:, :, :, D]
            nc.vector.tensor_copy(ones_dst, one_sb[:, 0:1].to_broadcast(ones_dst.shape))

            # ---------------- phase 1: k/v projections over full S ----------
            with tc.tile_pool(name="ph1w", bufs=1) as wp:
                wk_sb = wp.tile([P, NCC, F], DT_MM, tag="wk")
                wv_sb = wp.tile([P, NCC, F], DT_MM, tag="wv")
                # per-chunk DMAs spread across queues so the first matmul
                # isn't gated on a monolithic 2MB transfer
                for cc in range(NCC):
                    nc.sync.dma_start(wk_sb[:, cc, :], wk_r[:, cc, :])
                    nc.sync.dma_start(wv_sb[:, cc, :], wv_r[:, cc, :])
                with (
                    tc.tile_pool(name="ph1s", bufs=4) as sp,
                    tc.tile_pool(name="ph1p", bufs=4, space="PSUM") as pp,
                ):
                    for sw in range(S // TCH):
                        psk = [pp.tile([P, TCH], FP32, tag="psk", name="psk") for _ in range(NFC)]
                        psv = [pp.tile([P, TCH], FP32, tag="psv", name="psv") for _ in range(4)]
                        for cc in range(NCC):
                            xk_t = sp.tile([P, TCH], DT_MM, tag="xk")
                            xv_t = sp.tile([P, TCH], DT_MM, tag="xv")
                            nc.sync.dma_start(xk_t[:], xk_r[:, cc, sw * TCH:(sw + 1) * TCH])
                            nc.sync.dma_start(xv_t[:], xv_r[:, cc, sw * TCH:(sw + 1) * TCH])
                            for fc in range(NFC):
                                nc.tensor.matmul(
                                    psk[fc][:],
                                    wk_sb[:, cc, fc * P:(fc + 1) * P],
                                    xk_t[:],
                                    start=(cc == 0), stop=(cc == NCC - 1))
                            for ss in range(4):
                                nc.tensor.matmul(
                                    psv[ss][:],
                                    xv_t[:, ss * P:(ss + 1) * P],
                                    wv_sb[:, cc, :],
                                    start=(cc == 0), stop=(cc == NCC - 1))
                        for fc in range(NFC):
                            nc.vector.tensor_scalar_add(
                                kT_sb[:, fc, sw * TCH:(sw + 1) * TCH],
                                psk[fc][:], bk_sb[:, fc:fc + 1])
                        for ss in range(4):
                            sc = sw * 4 + ss
                            dst = v_sb[:, sc, :].rearrange("p (h e) -> p h e", e=HW)[:, :, 0:D]
                            src = psv[ss][:].rearrange("p (h e) -> p h e", e=D)
                            nc.vector.tensor_copy(dst, src)

            for cc in range(NCC):
                nc.sync.dma_start(wq_sb[:, cc, :], wq_r[:, cc, :])
            for dc in range(NFC):
                nc.sync.dma_start(wo_sb[:, dc, :], wo_r[:, dc, :])

            # ---------------- phase 2: flat pipelined block stream ----------
            with (
                tc.tile_pool(name="mainb", bufs=1) as mb_,
                tc.tile_pool(name="mains", bufs=4) as ms,
                tc.tile_pool(name="xqp", bufs=1) as xqp,
                tc.tile_pool(name="maino", bufs=2) as mo,
                tc.tile_pool(name="pscore", bufs=2, space="PSUM") as pscp,
                tc.tile_pool(name="ppv", bufs=2, space="PSUM") as ppvp,
                tc.tile_pool(name="pgen", bufs=2, space="PSUM") as pgp,
            ):
                expT = mb_.tile([P, NSC, 2 * TCH], EXP_DT, tag="expT")
                qT_sb = [mb_.tile([P, NFC, TCH], DT_MM, tag=f"qT{i}",
                                  name=f"qT{i}") for i in range(2)]
                attnT = [mb_.tile([P, NFC, TCH], DT_MM, tag=f"attnT{i}",
                                  name=f"attnT{i}") for i in range(2)]
                if use_mask:
                    emk_r = emask  # (S, T) natural: s rows

                def scores_mm(k, sc, psc):
                    t, pr = divmod(k, NFC)
                    for h in range(2):
                        nc.tensor.matmul(
                            psc[:, h, :],
                            kT_sb[h * D:(h + 1) * D, pr, sc * P:(sc + 1) * P],
                            qT_sb[t % 2][h * D:(h + 1) * D, pr, :],
                            start=True, stop=True)

                def exp_step(k, sc, psc):
                    t, pr = divmod(k, NFC)
                    nc.scalar.activation(
                        expT[:, sc, :], psc[:].rearrange("p a b -> p (a b)"),
                        mybir.ActivationFunctionType.Exp, scale=SCALE)
                    if use_mask:
                        em_t = ms.tile([P, TCH], FP32, tag="emk")
                        nc.sync.dma_start(
                            em_t[:],
                            emk_r[sc * P:(sc + 1) * P, t * TCH:(t + 1) * TCH])
                        for h in range(2):
                            nc.vector.tensor_mul(
                                expT[:, sc, h * TCH:(h + 1) * TCH],
                                expT[:, sc, h * TCH:(h + 1) * TCH],
                                em_t[:])

                def pv_mm(k, sc, ppvs):
                    _, pr = divmod(k, NFC)
                    for h in range(2):
                        hh = pr * 2 + h
                        nc.tensor.matmul(
                            ppvs[h][:],
                            v_sb[:, sc, hh * HW:(hh + 1) * HW],
                            expT[:, sc, h * TCH:(h + 1) * TCH],
                            start=(sc == 0), stop=(sc == NSC - 1))

                def tail(k, ppvs):
                    # denominator recip chain + raw numerator copy for pair k
                    t, pr = divmod(k, NFC)
                    for h in range(2):
                        nc.vector.tensor_copy(
                            rr32_sb[0:1, h * TCH:(h + 1) * TCH],
                            ppvs[h][D:D + 1, :])
                    nc.vector.reciprocal_approx_fast(rr32b_sb[0:1, :],
                                                     rr32_sb[0:1, :])
                    nc.vector.tensor_copy(rrow_sb[k % 2][0:1, :],
                                          rr32b_sb[0:1, :])
                    for h in range(2):
                        nc.vector.tensor_copy(
                            attnT[t % 2][h * D:(h + 1) * D, pr, :],
                            ppvs[h][0:D, :])

                def normfinish(j):
                    # broadcast recip across partitions + in-place multiply
                    t, pr = divmod(j, NFC)
                    for h in range(2):
                        pbc = pgp.tile([D, TCH], FP32, tag="pgen")
                        nc.tensor.matmul(
                            pbc[:], e1_sb[:],
                            rrow_sb[j % 2][:, h * TCH:(h + 1) * TCH],
                            start=True, stop=True)
                        dst = attnT[t % 2][h * D:(h + 1) * D, pr, :]
                        nc.vector.tensor_mul(dst, dst, pbc[:])

                def qproj_dma(t):
                    # stage the full t-chunk of xq in SBUF once; all four
                    # fc waves read it from there
                    xq_t = xqp.tile([P, NCC, TCH], DT_MM, tag="xqall")
                    for cc in range(NCC):
                        nc.sync.dma_start(xq_t[:, cc, :],
                                          xq_r[:, cc, t * TCH:(t + 1) * TCH])
                    return xq_t

                def qproj_cc2(fc, cc, psq, xq_t):
                    # two contraction steps of the single-fc wave
                    for c2 in (cc, cc + 1):
                        nc.tensor.matmul(
                            psq[:],
                            wq_sb[:, c2, fc * P:(fc + 1) * P],
                            xq_t[:, c2, :],
                            start=(c2 == 0), stop=(c2 == NCC - 1))

                def qproj_fin(t, fc, psq):
                    nc.vector.tensor_scalar_add(
                        qT_sb[t % 2][:, fc, :], psq[:],
                        bq_sb[:, fc:fc + 1])

                def outproj_group(t, g):
                    tw, fh = divmod(g, 2)
                    po = pgp.tile([P, TCH], FP32, tag="pgen")
                    for dc in range(NFC):
                        nc.tensor.matmul(
                            po[:],
                            attnT[t % 2][:, dc, tw * P:(tw + 1) * P],
                            wo_sb[:, dc, fh * TCH:(fh + 1) * TCH],
                            start=(dc == 0), stop=(dc == NFC - 1))
                    ob = mo.tile([P, TCH], FP32, tag="ob")
                    nc.vector.tensor_copy(ob[:], po[:])
                    nc.sync.dma_start(
                        out[t * TCH + tw * P: t * TCH + (tw + 1) * P,
                            fh * TCH:(fh + 1) * TCH],
                        ob[:])

                def qproj_full(t):
                    # unpipelined q projection (warmup for tcx 0)
                    xq_t = qproj_dma(t)
                    for fc in range(NFC):
                        psq = pgp.tile([P, TCH], FP32, tag="pgen", name="psq")
                        for cc in range(0, NCC, 2):
                            qproj_cc2(fc, cc, psq, xq_t)
                        qproj_fin(t, fc, psq)

                qproj_full(0)
                for k in range(NBLK + 2):
                    if k >= 2:
                        normfinish(k - 2)
                    qp = QPLAN.get(k)
                    op = OPLAN.get(k)
                    ppvs = None
                    if 1 <= k <= NBLK:
                        ppvs = [ppvp.tile([HW, TCH], FP32, tag="ppv",
                                          name="ppv") for _ in range(2)]
                    if k <= NBLK - 1:
                        # main interleaved stream
                        if qp is not None:
                            qt = qp[0]
                            xq_t = qproj_dma(qt)
                        psq = None
                        for sc in range(NSC):
                            psc = pscp.tile([P, 2, TCH], FP32, tag="pscore")
                            scores_mm(k, sc, psc)
                            if ppvs is not None:
                                pv_mm(k - 1, sc, ppvs)
                            exp_step(k, sc, psc)
                            if qp is not None:
                                # one fc wave per 4 s-chunks, 2 cc steps/chunk
                                fc, ph = divmod(sc, 4)
                                if ph == 0:
                                    psq = pgp.tile([P, TCH], FP32,
                                                   tag="pgen", name="psq")
                                qproj_cc2(fc, ph * 2, psq, xq_t)
                                if ph == 3:
                                    qproj_fin(qt, fc, psq)
                            if op is not None and sc % 4 == 3:
                                ot, groups = op
                                outproj_group(ot, groups[sc // 4])
                    elif k == NBLK:
                        # drain block: PV of the last pair only
                        for sc in range(NSC):
                            pv_mm(k - 1, sc, ppvs)
                    else:
                        # final block: outproj of the last t-chunk
                        ot, groups = OPLAN[k]
                        for g in groups:
                            outproj_group(ot, g)
                    if ppvs is not None:
                        tail(k - 1, ppvs)

    nc.compile()
    return nc


_CACHE = {}


def _get(use_mask: bool):
    if use_mask not in _CACHE:
        _CACHE[use_mask] = build(use_mask)
    return _CACHE[use_mask]


def kernel(query, key, value, attn_mask, key_padding_mask,
           Wq, bq, Wk, bk, Wv, bv, Wout, bout):
    global LAST_EXEC_NS, LAST_TRACE
    query = np.asarray(query, np.float32)
    key = np.asarray(key, np.float32)
    value = np.asarray(value, np.float32)
    attn_mask = np.asarray(attn_mask, np.float32)
    key_padding_mask = np.asarray(key_padding_mask)
    Wq, bq = np.asarray(Wq, np.float32), np.asarray(bq, np.float32)
    Wk, bk = np.asarray(Wk, np.float32), np.asarray(bk, np.float32)
    Wv, bv = np.asarray(Wv, np.float32), np.asarray(bv, np.float32)
    Wout, bout = np.asarray(Wout, np.float32), np.asarray(bout, np.float32)

    use_mask = bool(np.any(attn_mask)) or bool(np.any(key_padding_mask))
    nc = _get(use_mask)

    in_maps = []
    for c in range(NCORES):
        b, g = divmod(c, 2)
        gs = g * F
        im = {
            "xq": np.ascontiguousarray(query[b].T),
            "xk": np.ascontiguousarray(key[b].T),
            "xv": np.ascontiguousarray(value[b].T),
            "wq": np.ascontiguousarray(Wq[gs:gs + F, :].T),
            "wk": np.ascontiguousarray(Wk[gs:gs + F, :].T),
            "wv": np.ascontiguousarray(Wv[gs:gs + F, :].T),
            "wo": np.ascontiguousarray(Wout[:, gs:gs + F].T),
            "bqr": np.ascontiguousarray(bq[gs:gs + F].reshape(NFC, P).T),
            "bkr": np.ascontiguousarray(bk[gs:gs + F].reshape(NFC, P).T),
        }
        if use_mask:
            m = attn_mask.T.astype(np.float64).copy()
            m[key_padding_mask[b], :] = -np.inf
            im["emask"] = np.exp(m).astype(np.float32)
        in_maps.append(im)

    global LAST_NC, LAST_IN_MAPS
    LAST_NC, LAST_IN_MAPS = nc, in_maps
    res = run_bass_kernel_spmd(nc, in_maps, list(range(NCORES)))
    LAST_EXEC_NS = res.exec_time_ns
    LAST_TRACE = res.instructions_and_trace[1] if res.instructions_and_trace else None
    globals()["LAST_INSTS"] = (res.instructions_and_trace[0]
                               if res.instructions_and_trace else None)

    extra = (bv @ Wout.T + bout).astype(np.float32)
    outp = np.empty((B, T, C), np.float32)
    for b in range(B):
        outp[b] = res.results[2 * b]["out"] + res.results[2 * b + 1]["out"] + extra
    return outp


# revision 9
# speedup vs baseline: 1.0535x; 1.0535x over previous
"""Trainium2 Bass kernel for a dense fp32 MultiHeadAttention layer.

Problem (hardcoded): B=4, T=S=2048, C=1024, 16 heads x 64 dims, fp32.
  q = query @ Wq.T + bq ; k,v likewise
  scores = (q k^T) * D**-0.5 + attn_mask + padding_mask
  out = softmax(scores) @ v -> reshape -> @ Wout.T + bout

Sharding over 8 NeuronCores: core c = (batch b = c//2, head-group g = c%2).
Each core handles one batch and 8 of the 16 heads:
  - column-parallel q/k/v projections (512-dim slice of the projections)
  - attention for its 8 heads (full T x S, on-chip scores)
  - row-parallel out_proj producing a partial (T, C) output
Host sums the two partials per batch and adds the bias terms
(bout + bv @ Wout.T, which commutes with softmax since sum(weights)=1).

Layout notes (per core):
  - host ships transposed activations xT (C, T) in bf16 so projections
    need no on-device transposes and the (DMA-bound) preamble moves half
    the bytes; psum accumulation is fp32 so precision loss is input
    quantization only (~0.4%, tolerance is 2e-2):
      qT/kT (f-major): psum = wT_chunk.T @ xT_chunk   (f on partitions)
      v (s-major):     psum = xT_chunk.T @ wT_chunk   (s on partitions)
  - scores are computed transposed, (s on partitions, t free):
      psc = kT_chunk.T @ qT   so softmax's s-reduction becomes a matmul
  - v is stored with a ones column per head (65 wide); the PV matmul
      outT = [v|1].T @ exp(scores^T)
    then yields numerator rows 0..63 and the softmax denominator in row 64.
  - normalization: DVE recip of row 64, GpSimd partition_broadcast to a
    [128, 512] tile (both heads' bands), one DVE multiply.

Schedule: the Scalar engine (exp over all T*S*8head scores, ~1.1us per
1024-wide chunk, ~285us total) is the pacing engine; everything else is
arranged so Tensor/DVE/Pool work streams under its shadow.
  - preamble: q-projection of t-chunk 0, then k-projection s-window
    passes with pair-0 score/exp chunks issued diagonally as soon as
    their kT window lands, v-projection passes interleaved with the
    remaining pair-0 scores.
  - main stream: 16 head-pair "blocks"; block k interleaves, per
    s-chunk: scores(k, sc) -> PV(k-1, sc) -> exp(k, sc), so the Tensor
    engine always has exp-independent work.  exp writes land in a
    rolling 18-slot expT ring ((sc - 2k) mod 18), giving 2 s-chunk
    steps of write-after-read slack between exp and the previous
    pair's PV reads.
  - softmax normalization for pair k-1 is split: denominator recip +
    raw attnT copy at block-k tail (frees the PV psum immediately);
    partition-broadcast + multiply at block-k+1 head, so no engine
    waits on the DVE recip latency.
  - q/out projections are spread into fixed blocks (2/6/10 and
    5,7/9,11/13,14/17) that never mix in one block, keeping the 2-slot
    pgen PSUM ring deadlock-free.
"""

import os
import numpy as np
from ml_dtypes import bfloat16 as np_bf16

import concourse.bass as bass
import concourse.mybir as mybir
import concourse.tile as tile
from concourse import bacc
from concourse.bass_utils import run_bass_kernel_spmd

# ---- problem constants ----
B, T, S, C = 4, 2048, 2048, 1024
H, D = 16, 64
NCORES = 8
F = 512            # per-core projection slice (8 heads x 64)
SCALE = D ** -0.5
P = 128
TCH = 512          # t-chunk (score free dim)
NTC = T // TCH     # 4
NSC = S // P       # 16 s-chunks
NFC = F // P       # 4 f-chunks per core
NCC = C // P       # 8 contraction chunks
HW = 65            # v width per head incl. ones column
NBLK = NTC * NFC   # 16 head-pair blocks
NES = NSC + 2      # expT ring slots

FP32 = mybir.dt.float32
BF16 = mybir.dt.bfloat16

# matmul dtype for the fp32-precision operands (kT/qT/attnT/wo)
MM_DT = getattr(mybir.dt, os.environ.get("MHA_MM_DT", "float32r"))

LAST_EXEC_NS = None
LAST_TRACE = None
LAST_NC = None
LAST_IN_MAPS = None


DT_MM = MM_DT

# block -> target tcx for the q projection; never in an outproj block
QPLAN = {2: 1, 6: 2, 10: 3}
# block -> (tcx, [groups]) for the out projection (group g: tw=g//2, fh=g%2)
OPLAN = {5: (0, [0, 1, 2, 3]), 7: (0, [4, 5, 6, 7]),
         9: (1, [0, 1, 2, 3]), 11: (1, [4, 5, 6, 7]),
         13: (2, [0, 1, 2, 3]), 14: (2, [4, 5, 6, 7]),
         17: (3, [0, 1, 2, 3, 4, 5, 6, 7])}


def esl(k, sc):
    """expT ring slot for pair k, s-chunk sc (2 steps of WAR slack)."""
    return (sc - 2 * k) % NES


def build(use_mask: bool):
    nc = bacc.Bacc("TRN2", target_bir_lowering=False, debug=False,
                   num_devices=NCORES)

    EXP_DT = DT_MM if use_mask else BF16

    xq = nc.dram_tensor("xq", [C, T], BF16, kind="ExternalInput")
    xk = nc.dram_tensor("xk", [C, S], BF16, kind="ExternalInput")
    xv = nc.dram_tensor("xv", [C, S], BF16, kind="ExternalInput")
    wq = nc.dram_tensor("wq", [C, F], BF16, kind="ExternalInput")
    wk = nc.dram_tensor("wk", [C, F], BF16, kind="ExternalInput")
    wv = nc.dram_tensor("wv", [C, F], BF16, kind="ExternalInput")
    wo = nc.dram_tensor("wo", [F, C], DT_MM, kind="ExternalInput")
    bqr = nc.dram_tensor("bqr", [P, NFC], FP32, kind="ExternalInput")
    bkr = nc.dram_tensor("bkr", [P, NFC], FP32, kind="ExternalInput")
    if use_mask:
        emask = nc.dram_tensor("emask", [S, T], FP32, kind="ExternalInput")
    out = nc.dram_tensor("out", [T, C], FP32, kind="ExternalOutput")

    xq_r = xq.rearrange("(cc p) t -> p cc t", p=P)
    xk_r = xk.rearrange("(cc p) s -> p cc s", p=P)
    xv_r = xv.rearrange("(cc p) s -> p cc s", p=P)
    wq_r = wq.rearrange("(cc p) f -> p cc f", p=P)
    wk_r = wk.rearrange("(cc p) f -> p cc f", p=P)
    wv_r = wv.rearrange("(cc p) f -> p cc f", p=P)
    wo_r = wo.rearrange("(dc p) f -> p dc f", p=P)

    with tile.TileContext(nc) as tc:
        with (
            tc.tile_pool(name="const", bufs=1) as cp,
            tc.tile_pool(name="mains", bufs=4) as ms,
            tc.tile_pool(name="xqp", bufs=1) as xqp,
            tc.tile_pool(name="maino", bufs=2) as mo,
            tc.tile_pool(name="pscore", bufs=2, space="PSUM") as pscp,
        ):
            wq_sb = cp.tile([P, NCC, F], BF16, tag="wq")
            wo_sb = cp.tile([P, NFC, C], DT_MM, tag="wo")
            bq_sb = cp.tile([P, NFC], FP32, tag="bq")
            bk_sb = cp.tile([P, NFC], FP32, tag="bk")
            rr32_sb = cp.tile([1, 2 * TCH], FP32, tag="rr32")
            rr32b_sb = cp.tile([1, 2 * TCH], FP32, tag="rr32b")
            rbc_sb = cp.tile([P, 2 * TCH], FP32, tag="rbc")
            kT_sb = cp.tile([P, NFC, S], DT_MM, tag="kT")
            v_sb = cp.tile([P, NSC, 8 * HW], EXP_DT, tag="v")
            expT = cp.tile([P, NES, 2 * TCH], EXP_DT, tag="expT")
            qT_sb = [cp.tile([P, NFC, TCH], DT_MM, tag=f"qT{i}",
                             name=f"qT{i}") for i in range(2)]
            attnT = [cp.tile([P, NFC, TCH], DT_MM, tag=f"attnT{i}",
                             name=f"attnT{i}") for i in range(2)]

            nc.sync.dma_start(bq_sb[:], bqr[:])
            nc.sync.dma_start(bk_sb[:], bkr[:])
            for cc in range(NCC):
                nc.sync.dma_start(wq_sb[:, cc, :], wq_r[:, cc, :])
            # v ones column: bf16/fp32r tiles can't be memset directly;
            # broadcast-copy from an fp32 scratch column (exact for 0/1).
            one_sb = cp.tile([P, 1], FP32, tag="one")
            nc.any.memset(one_sb[:], 1.0)
            ones_dst = v_sb[:].rearrange("p s (h e) -> p s h e", e=HW)[:, :, :, D]
            nc.vector.tensor_copy(ones_dst, one_sb[:, 0:1].to_broadcast(ones_dst.shape))

            if use_mask:
                emk_r = emask  # (S, T) natural: s rows

            def scores_mm(k, sc, psc):
                t, pr = divmod(k, NFC)
                for h in range(2):
                    nc.tensor.matmul(
                        psc[:, h, :],
                        kT_sb[h * D:(h + 1) * D, pr, sc * P:(sc + 1) * P],
                        qT_sb[t % 2][h * D:(h + 1) * D, pr, :],
                        start=True, stop=True)

            def exp_step(k, sc, psc):
                t, pr = divmod(k, NFC)
                sl = esl(k, sc)
                nc.scalar.activation(
                    expT[:, sl, :], psc[:].rearrange("p a b -> p (a b)"),
                    mybir.ActivationFunctionType.Exp, scale=SCALE)
                if use_mask:
                    em_t = ms.tile([P, TCH], FP32, tag="emk")
                    nc.sync.dma_start(
                        em_t[:],
                        emk_r[sc * P:(sc + 1) * P, t * TCH:(t + 1) * TCH])
                    for h in range(2):
                        nc.vector.tensor_mul(
                            expT[:, sl, h * TCH:(h + 1) * TCH],
                            expT[:, sl, h * TCH:(h + 1) * TCH],
                            em_t[:])

            def score_step(k, sc):
                psc = pscp.tile([P, 2, TCH], FP32, tag="pscore")
                scores_mm(k, sc, psc)
                exp_step(k, sc, psc)
                return psc

            def pv_mm(k, sc, ppvs):
                _, pr = divmod(k, NFC)
                sl = esl(k, sc)
                for h in range(2):
                    hh = pr * 2 + h
                    nc.tensor.matmul(
                        ppvs[h][:],
                        v_sb[:, sc, hh * HW:(hh + 1) * HW],
                        expT[:, sl, h * TCH:(h + 1) * TCH],
                        start=(sc == 0), stop=(sc == NSC - 1))

            def tail(k, ppvs):
                # denominator recip chain + raw numerator copy for pair k
                t, pr = divmod(k, NFC)
                for h in range(2):
                    nc.vector.tensor_copy(
                        rr32_sb[0:1, h * TCH:(h + 1) * TCH],
                        ppvs[h][D:D + 1, :])
                nc.vector.reciprocal_approx_fast(rr32b_sb[0:1, :],
                                                 rr32_sb[0:1, :])
                for h in range(2):
                    nc.vector.tensor_copy(
                        attnT[t % 2][h * D:(h + 1) * D, pr, :],
                        ppvs[h][0:D, :])

            def normfinish(j):
                # both heads' recip rows broadcast to all 128 partitions on
                # the idle GpSimd engine, then per-head column-sliced
                # multiplies whose operands share a base partition (an SBUF
                # tensor_tensor constraint).
                t, pr = divmod(j, NFC)
                nc.gpsimd.partition_broadcast(rbc_sb[:], rr32b_sb[0:1, :],
                                              channels=P)
                for h in range(2):
                    dst = attnT[t % 2][h * D:(h + 1) * D, pr, :]
                    nc.vector.tensor_mul(
                        dst, dst,
                        rbc_sb[h * D:(h + 1) * D, h * TCH:(h + 1) * TCH])

            def qproj_dma(t):
                xq_t = xqp.tile([P, NCC, TCH], BF16, tag="xqall")
                for cc in range(NCC):
                    nc.sync.dma_start(xq_t[:, cc, :],
                                      xq_r[:, cc, t * TCH:(t + 1) * TCH])
                return xq_t

            def qproj_cc2(fc, cc, psq, xq_t):
                for c2 in (cc, cc + 1):
                    nc.tensor.matmul(
                        psq[:],
                        wq_sb[:, c2, fc * P:(fc + 1) * P],
                        xq_t[:, c2, :],
                        start=(c2 == 0), stop=(c2 == NCC - 1))

            def qproj_fin(t, fc, psq):
                nc.vector.tensor_scalar_add(
                    qT_sb[t % 2][:, fc, :], psq[:],
                    bq_sb[:, fc:fc + 1])

            # ---------------- phase 1: q/k/v proj + pair-0 scores -----------
            with (
                tc.tile_pool(name="ph1w", bufs=1) as wp,
                tc.tile_pool(name="ph1s", bufs=4) as sp,
                tc.tile_pool(name="ph1p", bufs=4, space="PSUM") as pp,
            ):
                wk_sb = wp.tile([P, NCC, F], BF16, tag="wk")
                wv_sb = wp.tile([P, NCC, F], BF16, tag="wv")
                for cc in range(NCC):
                    nc.sync.dma_start(wk_sb[:, cc, :], wk_r[:, cc, :])
                    nc.sync.dma_start(wv_sb[:, cc, :], wv_r[:, cc, :])

                # q projection of t-chunk 0 (pgen pool not open yet: use
                # the phase-1 psum tag)
                xq_t0 = qproj_dma(0)
                for fc in range(NFC):
                    psq = pp.tile([P, TCH], FP32, tag="pph", name="psq")
                    for cc in range(0, NCC, 2):
                        qproj_cc2(fc, cc, psq, xq_t0)
                    qproj_fin(0, fc, psq)

                for sw in range(S // TCH):
                    # k-pass for this s-window
                    psk = [pp.tile([P, TCH], FP32, tag="pph", name="psk")
                           for _ in range(NFC)]
                    for cc in range(NCC):
                        xk_t = sp.tile([P, TCH], BF16, tag="xk")
                        nc.sync.dma_start(
                            xk_t[:], xk_r[:, cc, sw * TCH:(sw + 1) * TCH])
                        for fc in range(NFC):
                            nc.tensor.matmul(
                                psk[fc][:],
                                wk_sb[:, cc, fc * P:(fc + 1) * P],
                                xk_t[:],
                                start=(cc == 0), stop=(cc == NCC - 1))
                    for fc in range(NFC):
                        nc.vector.tensor_scalar_add(
                            kT_sb[:, fc, sw * TCH:(sw + 1) * TCH],
                            psk[fc][:], bk_sb[:, fc:fc + 1])
                    # v-pass, with this window's pair-0 scores interleaved
                    psv = [pp.tile([P, TCH], FP32, tag="pph", name="psv")
                           for _ in range(4)]
                    for cc in range(NCC):
                        xv_t = sp.tile([P, TCH], BF16, tag="xv")
                        nc.sync.dma_start(
                            xv_t[:], xv_r[:, cc, sw * TCH:(sw + 1) * TCH])
                        for ss in range(4):
                            nc.tensor.matmul(
                                psv[ss][:],
                                xv_t[:, ss * P:(ss + 1) * P],
                                wv_sb[:, cc, :],
                                start=(cc == 0), stop=(cc == NCC - 1))
                        if cc % 2 == 1:
                            score_step(0, sw * 4 + cc // 2)
                    for ss in range(4):
                        sc = sw * 4 + ss
                        dst = v_sb[:, sc, :].rearrange(
                            "p (h e) -> p h e", e=HW)[:, :, 0:D]
                        src = psv[ss][:].rearrange("p (h e) -> p h e", e=D)
                        nc.vector.tensor_copy(dst, src)

            for dc in range(NFC):
                nc.sync.dma_start(wo_sb[:, dc, :], wo_r[:, dc, :])

            # ---------------- phase 2: flat pipelined block stream ----------
            with (
                tc.tile_pool(name="ppv", bufs=2, space="PSUM") as ppvp,
                tc.tile_pool(name="pgen", bufs=2, space="PSUM") as pgp,
            ):
                for k in range(1, NBLK + 2):
                    if k >= 2:
                        normfinish(k - 2)
                    qp = QPLAN.get(k)
                    op = OPLAN.get(k)
                    ppvs = None
                    if k <= NBLK:
                        ppvs = [ppvp.tile([HW, TCH], FP32, tag="ppv",
                                          name="ppv") for _ in range(2)]
                    if k <= NBLK - 1:
                        if qp is not None:
                            xq_t = qproj_dma(qp)
                        psq = None
                        for sc in range(NSC):
                            psc = pscp.tile([P, 2, TCH], FP32, tag="pscore")
                            scores_mm(k, sc, psc)
                            pv_mm(k - 1, sc, ppvs)
                            exp_step(k, sc, psc)
                            if qp is not None:
                                # one fc wave per 4 s-chunks, 2 cc steps each
                                fc, ph = divmod(sc, 4)
                                if ph == 0:
                                    psq = pgp.tile([P, TCH], FP32,
                                                   tag="pgen", name="psq")
                                qproj_cc2(fc, ph * 2, psq, xq_t)
                                if ph == 3:
                                    qproj_fin(qp, fc, psq)
                            if op is not None and sc % 4 == 3:
                                ot, groups = op
                                tw, fh = divmod(groups[sc // 4], 2)
                                po = pgp.tile([P, TCH], FP32, tag="pgen",
                                              name="po")
                                for dc in range(NFC):
                                    nc.tensor.matmul(
                                        po[:],
                                        attnT[ot % 2][:, dc, tw * P:(tw + 1) * P],
                                        wo_sb[:, dc, fh * TCH:(fh + 1) * TCH],
                                        start=(dc == 0), stop=(dc == NFC - 1))
                                ob = mo.tile([P, TCH], FP32, tag="ob")
                                nc.vector.tensor_copy(ob[:], po[:])
                                nc.sync.dma_start(
                                    out[ot * TCH + tw * P:
                                        ot * TCH + (tw + 1) * P,
                                        fh * TCH:(fh + 1) * TCH],
                                    ob[:])
                    elif k == NBLK:
                        for sc in range(NSC):
                            pv_mm(k - 1, sc, ppvs)
                    else:
                        ot, groups = OPLAN[k]
                        for g in groups:
                            tw, fh = divmod(g, 2)
                            po = pgp.tile([P, TCH], FP32, tag="pgen",
                                          name="po")
                            for dc in range(NFC):
                                nc.tensor.matmul(
                                    po[:],
                                    attnT[ot % 2][:, dc, tw * P:(tw + 1) * P],
                                    wo_sb[:, dc, fh * TCH:(fh + 1) * TCH],
                                    start=(dc == 0), stop=(dc == NFC - 1))
                            ob = mo.tile([P, TCH], FP32, tag="ob")
                            nc.vector.tensor_copy(ob[:], po[:])
                            nc.sync.dma_start(
                                out[ot * TCH + tw * P: ot * TCH + (tw + 1) * P,
                                    fh * TCH:(fh + 1) * TCH],
                                ob[:])
                    if ppvs is not None:
                        tail(k - 1, ppvs)

    nc.compile()
    return nc


_CACHE = {}


def _get(use_mask: bool):
    if use_mask not in _CACHE:
        _CACHE[use_mask] = build(use_mask)
    return _CACHE[use_mask]


def kernel(query, key, value, attn_mask, key_padding_mask,
           Wq, bq, Wk, bk, Wv, bv, Wout, bout):
    global LAST_EXEC_NS, LAST_TRACE
    query = np.asarray(query, np.float32)
    key = np.asarray(key, np.float32)
    value = np.asarray(value, np.float32)
    attn_mask = np.asarray(attn_mask, np.float32)
    key_padding_mask = np.asarray(key_padding_mask)
    Wq, bq = np.asarray(Wq, np.float32), np.asarray(bq, np.float32)
    Wk, bk = np.asarray(Wk, np.float32), np.asarray(bk, np.float32)
    Wv, bv = np.asarray(Wv, np.float32), np.asarray(bv, np.float32)
    Wout, bout = np.asarray(Wout, np.float32), np.asarray(bout, np.float32)

    use_mask = bool(np.any(attn_mask)) or bool(np.any(key_padding_mask))
    nc = _get(use_mask)

    in_maps = []
    for c in range(NCORES):
        b, g = divmod(c, 2)
        gs = g * F
        im = {
            "xq": np.ascontiguousarray(query[b].T).astype(np_bf16),
            "xk": np.ascontiguousarray(key[b].T).astype(np_bf16),
            "xv": np.ascontiguousarray(value[b].T).astype(np_bf16),
            "wq": np.ascontiguousarray(Wq[gs:gs + F, :].T).astype(np_bf16),
            "wk": np.ascontiguousarray(Wk[gs:gs + F, :].T).astype(np_bf16),
            "wv": np.ascontiguousarray(Wv[gs:gs + F, :].T).astype(np_bf16),
            "wo": np.ascontiguousarray(Wout[:, gs:gs + F].T),
            "bqr": np.ascontiguousarray(bq[gs:gs + F].reshape(NFC, P).T),
            "bkr": np.ascontiguousarray(bk[gs:gs + F].reshape(NFC, P).T),
        }
        if use_mask:
            m = attn_mask.T.astype(np.float64).copy()
            m[key_padding_mask[b], :] = -np.inf
            im["emask"] = np.exp(m).astype(np.float32)
        in_maps.append(im)

    global LAST_NC, LAST_IN_MAPS
    LAST_NC, LAST_IN_MAPS = nc, in_maps
    res = run_bass_kernel_spmd(nc, in_maps, list(range(NCORES)))
    LAST_EXEC_NS = res.exec_time_ns
    LAST_TRACE = res.instructions_and_trace[1] if res.instructions_and_trace else None
    globals()["LAST_INSTS"] = (res.instructions_and_trace[0]
                               if res.instructions_and_trace else None)

    extra = (bv @ Wout.T + bout).astype(np.float32)
    outp = np.empty((B, T, C), np.float32)
    for b in range(B):
        outp[b] = res.results[2 * b]["out"] + res.results[2 * b + 1]["out"] + extra
    return outp
